# revision 28
# baseline (speedup 1.0000x reference)
"""Trainium2 Bass kernel for cross-modal channel-attention fusion (CCDPA).

Math (per batch b):
  pooled[c,m,d] = mean_{w,h} x_m[b,c,d,w,h]
  q = Wq @ pooled[:,0,:] + bq ; k_m = Wk @ pooled[:,m,:] + bk
  a[c,m] = softmax_m(q[c]·k_m[c] / sqrt(D))
  out[b,o,s] = sum_m a[o,m] * (Wc[m] @ x_m[b,:,s] + bc[m,o])
             = sum_m (a[o,m]*Wc[m,o,:]) @ x_m[b,:,s]  + sum_m a[o,m]*bc[m,o]

Sharding: 8 cores = (batch b = p//2) x (d-half = p%2). Pass 1 streams the
shard once, computing per-(c,m,d) pooled sums; a tiny pairwise AllGather
exchanges the partner's d-half; attention weights are computed on-device;
pass 2 runs the 4 modality GEMMs with a-scaled weights accumulated in PSUM.

Perf structure:
- bf16 host-side conversion halves HBM traffic and runs the GEMM at full
  bf16 PE rate. Output is written bf16 and upconverted host-side.
- Pass-1 pooling reduces are split across DVE (scalar_tensor_tensor on
  slice halves, ~733ns/slice) and ACT (activation+accum_out, ~1240ns/slice)
  so pooling stays under the pass-1 DMA time. tensor_reduce (2.27us/slice
  effective) is avoided.
- z-precompute: during pass 1 the otherwise-idle PE runs the unscaled
  per-modality convs z_m = WcT_m @ x_m for the first 3 pairs (weights are
  known at start), with PSUM drains alternating DVE/ACT; pass 2 applies the
  exact attention weights with PE diagonal matmuls (out = sum_m
  diag(a_m) @ z_m + beff, half the PE cost of a GEMM pair and zero DVE), so
  those pairs need no re-read. Only 5 of 8 pairs are re-read.
- Pass-2 re-read DMAs are emitted right after pass 1 (program order =
  sequencer order) so they prefetch during the collective; pass 2 runs the
  z pairs first since their data is already resident, letting the PE start
  the moment the attention weights exist.
- 2-d-slice (512 KiB) DMA tiles amortize the ~0.6us/DMA DGE descriptor
  cost; input stream on the Sync ring, constants/collective staging/output
  on the ACT ring to avoid head-of-line blocking.

Measured on the 8-core axon TRN2 pod: ~291-307us warm (baseline f32
kernel: 735us). Phase split: ~130us pass 1 (DMA-bound at ~300-330
GB/s/core, z-GEMMs overlapped), ~35us collective+attention gap (partially
prefetch-filled), ~125us pass 2 (PE GEMM + re-read stream).
"""

from contextlib import ExitStack

import numpy as np
import ml_dtypes

import concourse.bacc as bacc
import concourse.bass as bass
import concourse.mybir as mybir
import concourse.tile as tile
from concourse.bass_utils import run_bass_kernel_spmd

F32 = mybir.dt.float32
BF16 = mybir.dt.bfloat16
NP_BF16 = ml_dtypes.bfloat16

B, C, D, W, H = 4, 256, 32, 32, 32
NCORES = 8
DHALF = D // 2  # d-slices per core
WH = W * H  # spatial elements per d-slice
S = DHALF * WH  # free elements per core shard


def _emit_program(nc, wh=WH, dhalf=DHALF):
    """Emit the SPMD per-core program. Identical on all 8 cores; per-core
    behavior comes only from per-core input data."""
    f32 = F32
    s = dhalf * wh
    dd = 2 * dhalf  # full D for this (possibly scaled-down) config
    nw = min(512, wh)  # matmul moving-dim chunk (1 PSUM bank)
    pair_w = 2 * wh  # two d-slices per DMA tile
    n_nh = pair_w // nw
    npairs = dhalf // 2
    zpairs = max(1, (3 * npairs) // 8)  # pairs conv'd during pass 1 (z-precompute)
    # z pairs are the FIRST pairs of pass 1; re-read pairs the rest
    AX = mybir.AxisListType.X
    AF = mybir.ActivationFunctionType
    ALU = mybir.AluOpType

    xs = [nc.dram_tensor(f"x{m}", [C, s], BF16, kind="ExternalInput") for m in range(4)]
    wqT_d = nc.dram_tensor("wqTaug", [dd + 1, dd], f32, kind="ExternalInput")
    wkT_d = nc.dram_tensor("wkTaug", [dd + 1, dd], f32, kind="ExternalInput")
    wc_d = nc.dram_tensor("wc", [4, C, C], f32, kind="ExternalInput")
    wcT_d = nc.dram_tensor("wcT", [4, C, C], BF16, kind="ExternalInput")
    bcT_d = nc.dram_tensor("bcT", [C, 4], f32, kind="ExternalInput")
    id_d = nc.dram_tensor("ident", [128, 128], f32, kind="ExternalInput")
    out_d = nc.dram_tensor("out", [C, s], BF16, kind="ExternalOutput")

    # pooling engine split by (ci, m): DVE takes g = ci*4+m < 5, ACT the rest
    def pool_engine(m, ci):
        return "dve" if ci * 4 + m < 5 else "act"

    with tile.TileContext(nc) as tc, ExitStack() as ctx:
        const = ctx.enter_context(tc.tile_pool(name="const", bufs=1))
        stream = ctx.enter_context(tc.tile_pool(name="stream", bufs=15))
        zpool = ctx.enter_context(tc.tile_pool(name="zpool", bufs=1))
        outp = ctx.enter_context(tc.tile_pool(name="outp", bufs=3))
        attn = ctx.enter_context(tc.tile_pool(name="attn", bufs=1))
        scr = ctx.enter_context(tc.tile_pool(name="scr", bufs=2))
        psM = ctx.enter_context(tc.tile_pool(name="psM", bufs=8, space="PSUM"))
        dramp = ctx.enter_context(tc.tile_pool(name="dramp", bufs=1, space="DRAM"))

        # ---- constant loads (off critical path) ----
        ident = const.tile([128, 128], f32, tag="ident", name="ident")
        nc.scalar.dma_start(out=ident[:], in_=id_d[:])
        wqT = const.tile([dd + 1, dd], f32, tag="wqT", name="wqT")
        nc.scalar.dma_start(out=wqT[:], in_=wqT_d[:])
        wkT = const.tile([dd + 1, dd], f32, tag="wkT", name="wkT")
        nc.scalar.dma_start(out=wkT[:], in_=wkT_d[:])
        wc_sb = []
        for oi in range(2):
            t = const.tile([128, 4 * C], f32, tag=f"wc{oi}", name=f"wc{oi}")
            for m in range(4):
                nc.scalar.dma_start(
                    out=t[:, m * C : (m + 1) * C],
                    in_=wc_d[m, oi * 128 : (oi + 1) * 128, :],
                )
            wc_sb.append(t)
        bc_sb = []
        for oi in range(2):
            t = const.tile([128, 4], f32, tag=f"bc{oi}", name=f"bc{oi}")
            nc.scalar.dma_start(out=t[:], in_=bcT_d[oi * 128 : (oi + 1) * 128, :])
            bc_sb.append(t)
        # plain transposed conv weights (bf16) for the pass-1 z-GEMMs:
        # wc_t[ci][c_local, m*C + oi*128 + o] = Wc[m, oi*128+o, ci*128+c]
        wc_t = []
        for ci in range(2):
            t = const.tile([128, 4 * C], BF16, tag=f"wct{ci}", name=f"wct{ci}")
            for m in range(4):
                for oi in range(2):
                    nc.scalar.dma_start(
                        out=t[:, m * C + oi * 128 : m * C + (oi + 1) * 128],
                        in_=wcT_d[
                            m, ci * 128 : (ci + 1) * 128, oi * 128 : (oi + 1) * 128
                        ],
                    )
            wc_t.append(t)

        # ---- pass 1: stream shard once, pooling sums per (c, m, d) ----
        # praw tiles per engine so DVE/ACT never co-write one tile:
        #   dve0: ci=0, all m  [128, 4*dhalf], col m*dhalf+d
        #   dve1: ci=1, m=0    [128, dhalf]
        #   act1: ci=1, m=1..3 [128, 3*dhalf], col (m-1)*dhalf+d
        praw_dve0 = attn.tile([128, 4 * dhalf], f32, tag="prd0", name="prd0")
        praw_dve1 = attn.tile([128, dhalf], f32, tag="prd1", name="prd1")
        praw_act1 = attn.tile([128, 3 * dhalf], f32, tag="pra1", name="pra1")
        junk_dve = attn.tile([128, wh // 2], BF16, tag="jkd", name="jkd")
        junk_act = attn.tile([128, wh], BF16, tag="jka", name="jka")

        def praw_slot(m, ci, d):
            if ci == 0:
                return praw_dve0[:, m * dhalf + d : m * dhalf + d + 1]
            if m == 0:
                return praw_dve1[:, d : d + 1]
            return praw_act1[:, (m - 1) * dhalf + d : (m - 1) * dhalf + d + 1]

        zt = {}
        zlist = list(range(zpairs))
        rest = list(range(zpairs, npairs))
        p1_order = []
        while zlist or rest:
            if zlist:
                p1_order.append(zlist.pop(0))
            if rest:
                p1_order.append(rest.pop(0))
        for p in p1_order:
            ptiles = {}
            for m in range(4):
                for ci in range(2):
                    t = stream.tile([128, pair_w], BF16, tag="xs", name="xs")
                    ptiles[(m, ci)] = t
                    nc.sync.dma_start(
                        out=t[:],
                        in_=xs[m][
                            ci * 128 : (ci + 1) * 128, p * pair_w : (p + 1) * pair_w
                        ],
                    )
                    for sub in range(2):
                        d = 2 * p + sub
                        acc = praw_slot(m, ci, d)
                        if pool_engine(m, ci) == "dve":
                            nc.vector.scalar_tensor_tensor(
                                out=junk_dve[:],
                                in0=t[:, sub * wh : sub * wh + wh // 2],
                                scalar=1.0,
                                in1=t[:, sub * wh + wh // 2 : (sub + 1) * wh],
                                op0=ALU.mult,
                                op1=ALU.add,
                                accum_out=acc,
                            )
                        else:
                            nc.scalar.activation(
                                junk_act[:],
                                t[:, sub * wh : (sub + 1) * wh],
                                AF.Copy,
                                accum_out=acc,
                            )
            if p < zpairs:
                # z-precompute on the otherwise-idle PE: z_m = WcT_m @ x_m
                # (unscaled conv; the exact attention weights are applied in
                # the pass-2 z-combine). PSUM drains alternate DVE/ACT.
                for m in range(4):
                    for oi in range(2):
                        z = zpool.tile(
                            [128, pair_w], BF16, tag=f"z{p}_{m}_{oi}",
                            name=f"z{p}_{m}_{oi}",
                        )
                        zt[(p, m, oi)] = z
                        for nh in range(n_nh):
                            ps = psM.tile([128, nw], f32, tag="ps", name="ps")
                            for ci in range(2):
                                nc.tensor.matmul(
                                    ps[:],
                                    lhsT=wc_t[ci][
                                        :, m * C + oi * 128 : m * C + (oi + 1) * 128
                                    ],
                                    rhs=ptiles[(m, ci)][:, nh * nw : (nh + 1) * nw],
                                    start=(ci == 0),
                                    stop=(ci == 1),
                                )
                            zsl = z[:, nh * nw : (nh + 1) * nw]
                            if nh % 2 == 0:
                                nc.vector.tensor_copy(zsl, ps[:])
                            else:
                                nc.scalar.activation(zsl, ps[:], AF.Copy)

        # ---- pass-2 re-read DMAs, emitted right after pass 1 so their only
        # dependency is stream-slot rotation: the slots map back to pairs
        # whose pooling finishes with the pass-1 stream, so the first ~14
        # prefetch through the collective+attention gap. ----
        xt2 = {}
        for p in range(zpairs, npairs):
            for m in range(4):
                for ci in range(2):
                    t = stream.tile([128, pair_w], BF16, tag="xs", name="xs")
                    nc.sync.dma_start(
                        out=t[:],
                        in_=xs[m][
                            ci * 128 : (ci + 1) * 128, p * pair_w : (p + 1) * pair_w
                        ],
                    )
                    xt2[(p, m, ci)] = t

        # ---- exchange pooled halves with the partner core (ACT ring) ----
        cc_in = dramp.tile([C, 4 * dhalf], f32, tag="cc_in", name="cc_in")
        cc_out = dramp.tile([2 * C, 4 * dhalf], f32, tag="cc_out", name="cc_out")
        nc.scalar.dma_start(out=cc_in[0:128, :], in_=praw_dve0[:])
        nc.scalar.dma_start(out=cc_in[128:256, 0:dhalf], in_=praw_dve1[:])
        nc.scalar.dma_start(out=cc_in[128:256, dhalf : 4 * dhalf], in_=praw_act1[:])
        nc.gpsimd.collective_compute(
            "AllGather",
            mybir.AluOpType.bypass,
            replica_groups=[[0, 1], [2, 3], [4, 5], [6, 7]],
            ins=[cc_in.opt()],
            outs=[cc_out.opt()],
        )
        # pooled[k][c_local, m*dd + h*dhalf + d] gathered with one 3D DMA per (k,h)
        pooled = [
            attn.tile([128, 4 * dd], f32, tag=f"pool{k}", name=f"pool{k}")
            for k in range(2)
        ]
        for k in range(2):
            for h in range(2):
                nc.scalar.dma_start(
                    out=pooled[k]
                    .rearrange("p (m z) -> p m z", m=4)[:, :, h * dhalf : (h + 1) * dhalf],
                    in_=cc_out[h * C + k * 128 : h * C + (k + 1) * 128, :]
                    .rearrange("p (m z) -> p m z", m=4),
                )

        # ---- attention weights ----
        # PTaug[m]: [D+1, 256] = pooled sums transposed, plus a ones-row
        ptaug = [
            attn.tile([dd + 1, C], f32, tag=f"pt{m}", name=f"pt{m}") for m in range(4)
        ]
        for m in range(4):
            nc.vector.memset(ptaug[m][:], 1.0)
            for k in range(2):
                pst = psM.tile([128, nw], f32, tag="ps", name="ps")
                nc.tensor.transpose(
                    pst[0:dd, 0:128], pooled[k][:, m * dd : (m + 1) * dd], ident[:]
                )
                nc.vector.tensor_copy(
                    ptaug[m][0:dd, k * 128 : (k + 1) * 128], pst[0:dd, 0:128]
                )
        qc = []
        kcs = [[None] * 2 for _ in range(4)]
        for k in range(2):
            psq = psM.tile([128, nw], f32, tag="ps", name="ps")
            nc.tensor.matmul(
                psq[:, 0:dd], lhsT=ptaug[0][:, k * 128 : (k + 1) * 128], rhs=wqT[:],
                start=True, stop=True,
            )
            t = attn.tile([128, dd], f32, tag=f"qc{k}", name=f"qc{k}")
            nc.vector.tensor_copy(t[:], psq[:, 0:dd])
            qc.append(t)
            for m in range(4):
                psk = psM.tile([128, nw], f32, tag="ps", name="ps")
                nc.tensor.matmul(
                    psk[:, 0:dd], lhsT=ptaug[m][:, k * 128 : (k + 1) * 128], rhs=wkT[:],
                    start=True, stop=True,
                )
                tk = attn.tile([128, dd], f32, tag=f"kc{m}_{k}", name=f"kc{m}_{k}")
                nc.vector.tensor_copy(tk[:], psk[:, 0:dd])
                kcs[m][k] = tk
        # logits via one STT (mul+mul, accum=sum) per (k, m); softmax over m
        a_sb = []
        for k in range(2):
            lg = attn.tile([128, 4], f32, tag=f"lg{k}", name=f"lg{k}")
            jq = scr.tile([128, dd], f32, tag="ttr", name="ttr")
            for m in range(4):
                nc.vector.scalar_tensor_tensor(
                    out=jq[:], in0=qc[k][:], scalar=1.0, in1=kcs[m][k][:],
                    op0=ALU.mult, op1=ALU.mult, accum_out=lg[:, m : m + 1],
                )
            mx = attn.tile([128, 1], f32, tag=f"mx{k}", name=f"mx{k}")
            nc.vector.reduce_max(out=mx[:], in_=lg[:], axis=AX)
            nc.vector.tensor_scalar_sub(out=lg[:], in0=lg[:], scalar1=mx[:])
            ex = attn.tile([128, 4], f32, tag=f"ex{k}", name=f"ex{k}")
            nc.scalar.activation(ex[:], lg[:], AF.Exp)
            sm = attn.tile([128, 1], f32, tag=f"sm{k}", name=f"sm{k}")
            nc.vector.reduce_sum(out=sm[:], in_=ex[:], axis=AX)
            rc = attn.tile([128, 1], f32, tag=f"rc{k}", name=f"rc{k}")
            nc.vector.reciprocal(out=rc[:], in_=sm[:])
            at = attn.tile([128, 4], f32, tag=f"a{k}", name=f"a{k}")
            nc.vector.tensor_scalar_mul(out=at[:], in0=ex[:], scalar1=rc[:])
            a_sb.append(at)

        # ---- scaled weights: weff[oi] = a[:,m] * wc rows; wt = weff^T ----
        weff = [
            attn.tile([128, 4 * C], f32, tag=f"weff{oi}", name=f"weff{oi}")
            for oi in range(2)
        ]
        beff = []
        for oi in range(2):
            for m in range(4):
                nc.vector.tensor_scalar_mul(
                    out=weff[oi][:, m * C : (m + 1) * C],
                    in0=wc_sb[oi][:, m * C : (m + 1) * C],
                    scalar1=a_sb[oi][:, m : m + 1],
                )
            bt = scr.tile([128, 4], f32, tag="btmp", name="btmp")
            be = attn.tile([128, 1], f32, tag=f"beff{oi}", name=f"beff{oi}")
            nc.vector.tensor_mul(bt[:], a_sb[oi][:], bc_sb[oi][:])
            nc.vector.reduce_sum(out=be[:], in_=bt[:], axis=AX)
            beff.append(be)
        wt_sb = [
            attn.tile([128, 4 * C], BF16, tag=f"wt{ci}", name=f"wt{ci}")
            for ci in range(2)
        ]
        for m in range(4):
            for oi in range(2):
                for ci in range(2):
                    psw = psM.tile([128, nw], f32, tag="ps", name="ps")
                    nc.tensor.transpose(
                        psw[:, 0:128],
                        weff[oi][:, m * C + ci * 128 : m * C + (ci + 1) * 128],
                        ident[:],
                    )
                    nc.vector.tensor_copy(
                        wt_sb[ci][:, m * C + oi * 128 : m * C + (oi + 1) * 128],
                        psw[:, 0:128],
                    )

        # ---- pass 2: out[o, s] = sum_{m,c} wt[c, o] * x_m[c, s] (+ beff) ----
        # cached pairs first (no DMA), streamed pairs re-read on the Sync ring
        # ---- diag(a_m) tiles (bf16) for the PE z-combines ----
        diag = {}
        for oi in range(2):
            for m in range(4):
                t = attn.tile([128, 128], BF16, tag=f"dg{oi}_{m}", name=f"dg{oi}_{m}")
                nc.vector.tensor_scalar(
                    out=t[:], in0=ident[:], scalar1=a_sb[oi][:, m : m + 1],
                    scalar2=0.0, op0=ALU.mult, op1=ALU.add,
                )
                diag[(oi, m)] = t

        # ---- pass 2 ----
        # z pairs first: their data is already in SBUF, so the PE starts
        # immediately once the attention weights exist while the re-read
        # prefetch keeps streaming for the GEMM pairs.
        def emit_zdiag(p):
            # out = sum_m diag(a_m) @ z_m (exact attention), beff on the drain
            for oi in range(2):
                ot = outp.tile([128, pair_w], BF16, tag="ot", name="ot")
                for nh in range(n_nh):
                    ps = psM.tile([128, nw], f32, tag="ps", name="ps")
                    for m in range(4):
                        nc.tensor.matmul(
                            ps[:], lhsT=diag[(oi, m)][:],
                            rhs=zt[(p, m, oi)][:, nh * nw : (nh + 1) * nw],
                            start=(m == 0), stop=(m == 3),
                        )
                    nc.vector.tensor_scalar_add(
                        out=ot[:, nh * nw : (nh + 1) * nw], in0=ps[:],
                        scalar1=beff[oi][:],
                    )
                nc.scalar.dma_start(
                    out=out_d[
                        oi * 128 : (oi + 1) * 128, p * pair_w : (p + 1) * pair_w
                    ],
                    in_=ot[:],
                )

        def emit_gemm(p):
            # out = wt^T x (+ beff) from re-read tiles
            xt = {(m, ci): xt2[(p, m, ci)] for m in range(4) for ci in range(2)}
            for oi in range(2):
                ot = outp.tile([128, pair_w], BF16, tag="ot", name="ot")
                for nh in range(n_nh):
                    ps = psM.tile([128, nw], f32, tag="ps", name="ps")
                    for m in range(4):
                        for ci in range(2):
                            nc.tensor.matmul(
                                ps[:],
                                lhsT=wt_sb[ci][
                                    :, m * C + oi * 128 : m * C + (oi + 1) * 128
                                ],
                                rhs=xt[(m, ci)][:, nh * nw : (nh + 1) * nw],
                                start=(m == 0 and ci == 0),
                                stop=(m == 3 and ci == 1),
                            )
                    nc.vector.tensor_scalar_add(
                        out=ot[:, nh * nw : (nh + 1) * nw], in0=ps[:],
                        scalar1=beff[oi][:],
                    )
                nc.scalar.dma_start(
                    out=out_d[
                        oi * 128 : (oi + 1) * 128, p * pair_w : (p + 1) * pair_w
                    ],
                    in_=ot[:],
                )

        for p in range(zpairs):
            emit_zdiag(p)
        for p in range(zpairs, npairs):
            emit_gemm(p)
    return nc


_CACHED = {}
LAST_RESULTS = None


def _build(wh=WH, dhalf=DHALF):
    key = (wh, dhalf)
    if key not in _CACHED:
        nc = bacc.Bacc(
            "TRN2",
            target_bir_lowering=False,
            debug=False,
            enable_asserts=False,
            num_devices=NCORES,
        )
        _emit_program(nc, wh=wh, dhalf=dhalf)
        nc.compile()
        _CACHED[key] = nc
    return _CACHED[key]


def _host_prep(Wq, bq, Wk, bk, bc, wh_pool, d):
    """Fold pooling mean + logit scale into augmented [D+1, D] q/k weights."""
    scale_q = 1.0 / (wh_pool * np.sqrt(np.float32(d)))
    wqTaug = np.concatenate(
        [(Wq * scale_q).T, (bq / np.sqrt(np.float32(d)))[None, :]], axis=0
    ).astype(np.float32)
    wkTaug = np.concatenate([(Wk / wh_pool).T, bk[None, :]], axis=0).astype(np.float32)
    bcT = np.ascontiguousarray(bc.T).astype(np.float32)
    ident = np.eye(128, dtype=np.float32)
    return wqTaug, wkTaug, bcT, ident


def kernel(m1, m2, m3, m4, Wq, bq, Wk, bk, Wc, bc, **run_kwargs):
    ms = [np.asarray(x, dtype=np.float32) for x in (m1, m2, m3, m4)]
    Wq, bq, Wk, bk, Wc, bc = (
        np.asarray(x, dtype=np.float32) for x in (Wq, bq, Wk, bk, Wc, bc)
    )
    nc = _build()
    wqTaug, wkTaug, bcT, ident = _host_prep(Wq, bq, Wk, bk, bc, WH, D)
    in_maps = []
    for p in range(NCORES):
        b, h = divmod(p, 2)
        im = {
            f"x{m}": np.ascontiguousarray(
                ms[m][b, :, h * DHALF : (h + 1) * DHALF]
            ).reshape(C, S).astype(NP_BF16)
            for m in range(4)
        }
        im.update(wqTaug=wqTaug, wkTaug=wkTaug, wc=Wc, bcT=bcT, ident=ident,
                  wcT=np.ascontiguousarray(Wc.transpose(0, 2, 1)).astype(NP_BF16))
        in_maps.append(im)
    global LAST_RESULTS
    res = run_bass_kernel_spmd(
        nc, in_maps, core_ids=list(range(NCORES)), **run_kwargs
    )
    LAST_RESULTS = res
    out = np.empty((B, C, D, W, H), np.float32)
    for p in range(NCORES):
        b, h = divmod(p, 2)
        out[b, :, h * DHALF : (h + 1) * DHALF] = (
            res.results[p]["out"].astype(np.float32).reshape(C, DHALF, W, H)
        )
    return out


# revision 30
# speedup vs baseline: 1.0320x; 1.0320x over previous
"""Trainium2 Bass kernel for cross-modal channel-attention fusion (CCDPA).

Math (per batch b):
  pooled[c,m,d] = mean_{w,h} x_m[b,c,d,w,h]
  q = Wq @ pooled[:,0,:] + bq ; k_m = Wk @ pooled[:,m,:] + bk
  a[c,m] = softmax_m(q[c]·k_m[c] / sqrt(D))
  out[b,o,s] = sum_m a[o,m] * (Wc[m] @ x_m[b,:,s] + bc[m,o])
             = sum_m (a[o,m]*Wc[m,o,:]) @ x_m[b,:,s]  + sum_m a[o,m]*bc[m,o]

Sharding: 8 cores = (batch b = p//2) x (d-half = p%2). Pass 1 streams the
shard once, computing per-(c,m,d) pooled sums; a tiny pairwise AllGather
exchanges the partner's d-half; attention weights are computed on-device;
pass 2 runs the 4 modality GEMMs with a-scaled weights accumulated in PSUM.

Perf structure:
- bf16 host-side conversion halves HBM traffic and runs the GEMM at full
  bf16 PE rate. Output is written bf16 and upconverted host-side.
- Pass-1 pooling reduces are split across DVE (scalar_tensor_tensor on
  slice halves, ~733ns/slice) and ACT (activation+accum_out, ~1240ns/slice)
  so pooling stays under the pass-1 DMA time. tensor_reduce (2.27us/slice
  effective) is avoided.
- z-precompute: during pass 1 the otherwise-idle PE runs the unscaled
  per-modality convs z_m = WcT_m @ x_m for the first 3 pairs (weights are
  known at start), with PSUM drains alternating DVE/ACT; pass 2 applies the
  exact attention weights with PE diagonal matmuls (out = sum_m
  diag(a_m) @ z_m + beff, half the PE cost of a GEMM pair and zero DVE), so
  those pairs need no re-read. Only 5 of 8 pairs are re-read.
- Pass-2 re-read DMAs are emitted right after pass 1 (program order =
  sequencer order) so they prefetch during the collective; pass 2 runs the
  z pairs first since their data is already resident, letting the PE start
  the moment the attention weights exist.
- 2-d-slice (512 KiB) DMA tiles amortize the ~0.6us/DMA DGE descriptor
  cost; input stream on the Sync ring, constants/collective staging/output
  on the ACT ring to avoid head-of-line blocking.

Measured on the 8-core axon TRN2 pod: ~291-307us warm (baseline f32
kernel: 735us). Phase split: ~130us pass 1 (DMA-bound at ~300-330
GB/s/core, z-GEMMs overlapped), ~35us collective+attention gap (partially
prefetch-filled), ~125us pass 2 (PE GEMM + re-read stream).
"""

from contextlib import ExitStack

import numpy as np
import ml_dtypes

import concourse.bacc as bacc
import concourse.bass as bass
import concourse.mybir as mybir
import concourse.tile as tile
from concourse.bass_utils import run_bass_kernel_spmd

F32 = mybir.dt.float32
BF16 = mybir.dt.bfloat16
NP_BF16 = ml_dtypes.bfloat16

B, C, D, W, H = 4, 256, 32, 32, 32
NCORES = 8
DHALF = D // 2  # d-slices per core
WH = W * H  # spatial elements per d-slice
S = DHALF * WH  # free elements per core shard


def _emit_program(nc, wh=WH, dhalf=DHALF):
    """Emit the SPMD per-core program. Identical on all 8 cores; per-core
    behavior comes only from per-core input data."""
    f32 = F32
    s = dhalf * wh
    dd = 2 * dhalf  # full D for this (possibly scaled-down) config
    nw = min(512, wh)  # matmul moving-dim chunk (1 PSUM bank)
    pair_w = 2 * wh  # two d-slices per DMA tile
    n_nh = pair_w // nw
    npairs = dhalf // 2
    zpairs = max(1, (3 * npairs) // 8)  # pairs conv'd during pass 1 (z-precompute)
    # z pairs are the FIRST pairs of pass 1; re-read pairs the rest
    AX = mybir.AxisListType.X
    AF = mybir.ActivationFunctionType
    ALU = mybir.AluOpType

    xs = [nc.dram_tensor(f"x{m}", [C, s], BF16, kind="ExternalInput") for m in range(4)]
    wqT_d = nc.dram_tensor("wqTaug", [dd + 1, dd], f32, kind="ExternalInput")
    wkT_d = nc.dram_tensor("wkTaug", [dd + 1, dd], f32, kind="ExternalInput")
    wc_d = nc.dram_tensor("wc", [4, C, C], f32, kind="ExternalInput")
    wcT_d = nc.dram_tensor("wcT", [4, C, C], BF16, kind="ExternalInput")
    bcT_d = nc.dram_tensor("bcT", [C, 4], f32, kind="ExternalInput")
    id_d = nc.dram_tensor("ident", [128, 128], f32, kind="ExternalInput")
    out_d = nc.dram_tensor("out", [C, s], BF16, kind="ExternalOutput")

    # pooling engine split by (ci, m): DVE takes g = ci*4+m < 5, ACT the rest
    def pool_engine(m, ci):
        return "dve" if ci * 4 + m < 5 else "act"

    with tile.TileContext(nc) as tc, ExitStack() as ctx:
        const = ctx.enter_context(tc.tile_pool(name="const", bufs=1))
        stream = ctx.enter_context(tc.tile_pool(name="stream", bufs=15))
        zpool = ctx.enter_context(tc.tile_pool(name="zpool", bufs=1))
        outp = ctx.enter_context(tc.tile_pool(name="outp", bufs=3))
        attn = ctx.enter_context(tc.tile_pool(name="attn", bufs=1))
        scr = ctx.enter_context(tc.tile_pool(name="scr", bufs=2))
        psM = ctx.enter_context(tc.tile_pool(name="psM", bufs=8, space="PSUM"))
        dramp = ctx.enter_context(tc.tile_pool(name="dramp", bufs=1, space="DRAM"))

        # ---- constant loads (off critical path) ----
        ident = const.tile([128, 128], f32, tag="ident", name="ident")
        nc.scalar.dma_start(out=ident[:], in_=id_d[:])
        wqT = const.tile([dd + 1, dd], f32, tag="wqT", name="wqT")
        nc.scalar.dma_start(out=wqT[:], in_=wqT_d[:])
        wkT = const.tile([dd + 1, dd], f32, tag="wkT", name="wkT")
        nc.scalar.dma_start(out=wkT[:], in_=wkT_d[:])
        wc_sb = []
        for oi in range(2):
            t = const.tile([128, 4 * C], f32, tag=f"wc{oi}", name=f"wc{oi}")
            for m in range(4):
                nc.scalar.dma_start(
                    out=t[:, m * C : (m + 1) * C],
                    in_=wc_d[m, oi * 128 : (oi + 1) * 128, :],
                )
            wc_sb.append(t)
        bc_sb = []
        for oi in range(2):
            t = const.tile([128, 4], f32, tag=f"bc{oi}", name=f"bc{oi}")
            nc.scalar.dma_start(out=t[:], in_=bcT_d[oi * 128 : (oi + 1) * 128, :])
            bc_sb.append(t)
        # plain transposed conv weights (bf16) for the pass-1 z-GEMMs:
        # wc_t[ci][c_local, m*C + oi*128 + o] = Wc[m, oi*128+o, ci*128+c]
        wc_t = []
        for ci in range(2):
            t = const.tile([128, 4 * C], BF16, tag=f"wct{ci}", name=f"wct{ci}")
            for m in range(4):
                for oi in range(2):
                    nc.scalar.dma_start(
                        out=t[:, m * C + oi * 128 : m * C + (oi + 1) * 128],
                        in_=wcT_d[
                            m, ci * 128 : (ci + 1) * 128, oi * 128 : (oi + 1) * 128
                        ],
                    )
            wc_t.append(t)

        # ---- pass 1: stream shard once, pooling sums per (c, m, d) ----
        # praw tiles per engine so DVE/ACT never co-write one tile:
        #   dve0: ci=0, all m  [128, 4*dhalf], col m*dhalf+d
        #   dve1: ci=1, m=0    [128, dhalf]
        #   act1: ci=1, m=1..3 [128, 3*dhalf], col (m-1)*dhalf+d
        praw_dve0 = attn.tile([128, 4 * dhalf], f32, tag="prd0", name="prd0")
        praw_dve1 = attn.tile([128, dhalf], f32, tag="prd1", name="prd1")
        praw_act1 = attn.tile([128, 3 * dhalf], f32, tag="pra1", name="pra1")
        junk_dve = attn.tile([128, wh // 2], BF16, tag="jkd", name="jkd")
        junk_act = attn.tile([128, wh], BF16, tag="jka", name="jka")

        def praw_slot(m, ci, d):
            if ci == 0:
                return praw_dve0[:, m * dhalf + d : m * dhalf + d + 1]
            if m == 0:
                return praw_dve1[:, d : d + 1]
            return praw_act1[:, (m - 1) * dhalf + d : (m - 1) * dhalf + d + 1]

        # ptaug tiles memset early: only the ones-row survives the transposed
        # copies, and doing it here keeps it off the post-collective path
        ptaug = [
            attn.tile([dd + 1, C], f32, tag=f"pt{m}", name=f"pt{m}") for m in range(4)
        ]
        for m in range(4):
            nc.vector.memset(ptaug[m][:], 1.0)

        zt = {}
        zlist = list(range(zpairs))
        rest = list(range(zpairs, npairs))
        p1_order = []
        while zlist or rest:
            if zlist:
                p1_order.append(zlist.pop(0))
            if rest:
                p1_order.append(rest.pop(0))
        for p in p1_order:
            ptiles = {}
            for m in range(4):
                for ci in range(2):
                    t = stream.tile([128, pair_w], BF16, tag="xs", name="xs")
                    ptiles[(m, ci)] = t
                    nc.sync.dma_start(
                        out=t[:],
                        in_=xs[m][
                            ci * 128 : (ci + 1) * 128, p * pair_w : (p + 1) * pair_w
                        ],
                    )
                    for sub in range(2):
                        d = 2 * p + sub
                        acc = praw_slot(m, ci, d)
                        if pool_engine(m, ci) == "dve":
                            nc.vector.scalar_tensor_tensor(
                                out=junk_dve[:],
                                in0=t[:, sub * wh : sub * wh + wh // 2],
                                scalar=1.0,
                                in1=t[:, sub * wh + wh // 2 : (sub + 1) * wh],
                                op0=ALU.mult,
                                op1=ALU.add,
                                accum_out=acc,
                            )
                        else:
                            nc.scalar.activation(
                                junk_act[:],
                                t[:, sub * wh : (sub + 1) * wh],
                                AF.Copy,
                                accum_out=acc,
                            )
            if p < zpairs:
                # z-precompute on the otherwise-idle PE: z_m = WcT_m @ x_m
                # (unscaled conv; the exact attention weights are applied in
                # the pass-2 z-combine). PSUM drains alternate DVE/ACT.
                for m in range(4):
                    for oi in range(2):
                        z = zpool.tile(
                            [128, pair_w], BF16, tag=f"z{p}_{m}_{oi}",
                            name=f"z{p}_{m}_{oi}",
                        )
                        zt[(p, m, oi)] = z
                        for nh in range(n_nh):
                            ps = psM.tile([128, nw], f32, tag="ps", name="ps")
                            for ci in range(2):
                                nc.tensor.matmul(
                                    ps[:],
                                    lhsT=wc_t[ci][
                                        :, m * C + oi * 128 : m * C + (oi + 1) * 128
                                    ],
                                    rhs=ptiles[(m, ci)][:, nh * nw : (nh + 1) * nw],
                                    start=(ci == 0),
                                    stop=(ci == 1),
                                )
                            zsl = z[:, nh * nw : (nh + 1) * nw]
                            if nh % 2 == 0:
                                nc.vector.tensor_copy(zsl, ps[:])
                            else:
                                nc.scalar.activation(zsl, ps[:], AF.Copy)

        # ---- pass-2 re-read DMAs, emitted right after pass 1 so their only
        # dependency is stream-slot rotation: the slots map back to pairs
        # whose pooling finishes with the pass-1 stream, so the first ~14
        # prefetch through the collective+attention gap. ----
        xt2 = {}
        for p in range(zpairs, npairs):
            for m in range(4):
                for ci in range(2):
                    t = stream.tile([128, pair_w], BF16, tag="xs", name="xs")
                    nc.sync.dma_start(
                        out=t[:],
                        in_=xs[m][
                            ci * 128 : (ci + 1) * 128, p * pair_w : (p + 1) * pair_w
                        ],
                    )
                    xt2[(p, m, ci)] = t

        # ---- exchange pooled halves with the partner core (ACT ring) ----
        cc_in = dramp.tile([C, 4 * dhalf], f32, tag="cc_in", name="cc_in")
        cc_out = dramp.tile([2 * C, 4 * dhalf], f32, tag="cc_out", name="cc_out")
        nc.scalar.dma_start(out=cc_in[0:128, :], in_=praw_dve0[:])
        nc.scalar.dma_start(out=cc_in[128:256, 0:dhalf], in_=praw_dve1[:])
        nc.scalar.dma_start(out=cc_in[128:256, dhalf : 4 * dhalf], in_=praw_act1[:])
        nc.gpsimd.collective_compute(
            "AllGather",
            mybir.AluOpType.bypass,
            replica_groups=[[0, 1], [2, 3], [4, 5], [6, 7]],
            ins=[cc_in.opt()],
            outs=[cc_out.opt()],
        )
        # pooled[k][c_local, m*dd + h*dhalf + d] gathered with one 3D DMA per (k,h)
        pooled = [
            attn.tile([128, 4 * dd], f32, tag=f"pool{k}", name=f"pool{k}")
            for k in range(2)
        ]
        for k in range(2):
            for h in range(2):
                nc.scalar.dma_start(
                    out=pooled[k]
                    .rearrange("p (m z) -> p m z", m=4)[:, :, h * dhalf : (h + 1) * dhalf],
                    in_=cc_out[h * C + k * 128 : h * C + (k + 1) * 128, :]
                    .rearrange("p (m z) -> p m z", m=4),
                )

        # ---- attention weights ----
        # PTaug[m]: [D+1, 256] = pooled sums transposed, plus a ones-row
        for m in range(4):
            for k in range(2):
                pst = psM.tile([128, nw], f32, tag="ps", name="ps")
                nc.tensor.transpose(
                    pst[0:dd, 0:128], pooled[k][:, m * dd : (m + 1) * dd], ident[:]
                )
                nc.vector.tensor_copy(
                    ptaug[m][0:dd, k * 128 : (k + 1) * 128], pst[0:dd, 0:128]
                )
        qc = []
        kcs = [[None] * 2 for _ in range(4)]
        for k in range(2):
            psq = psM.tile([128, nw], f32, tag="ps", name="ps")
            nc.tensor.matmul(
                psq[:, 0:dd], lhsT=ptaug[0][:, k * 128 : (k + 1) * 128], rhs=wqT[:],
                start=True, stop=True,
            )
            t = attn.tile([128, dd], f32, tag=f"qc{k}", name=f"qc{k}")
            nc.vector.tensor_copy(t[:], psq[:, 0:dd])
            qc.append(t)
            for m in range(4):
                psk = psM.tile([128, nw], f32, tag="ps", name="ps")
                nc.tensor.matmul(
                    psk[:, 0:dd], lhsT=ptaug[m][:, k * 128 : (k + 1) * 128], rhs=wkT[:],
                    start=True, stop=True,
                )
                tk = attn.tile([128, dd], f32, tag=f"kc{m}_{k}", name=f"kc{m}_{k}")
                nc.vector.tensor_copy(tk[:], psk[:, 0:dd])
                kcs[m][k] = tk
        # logits via one STT (mul+mul, accum=sum) per (k, m); softmax over m
        a_sb = []
        for k in range(2):
            lg = attn.tile([128, 4], f32, tag=f"lg{k}", name=f"lg{k}")
            jq = scr.tile([128, dd], f32, tag="ttr", name="ttr")
            for m in range(4):
                nc.vector.scalar_tensor_tensor(
                    out=jq[:], in0=qc[k][:], scalar=1.0, in1=kcs[m][k][:],
                    op0=ALU.mult, op1=ALU.mult, accum_out=lg[:, m : m + 1],
                )
            mx = attn.tile([128, 1], f32, tag=f"mx{k}", name=f"mx{k}")
            nc.vector.reduce_max(out=mx[:], in_=lg[:], axis=AX)
            nc.vector.tensor_scalar_sub(out=lg[:], in0=lg[:], scalar1=mx[:])
            ex = attn.tile([128, 4], f32, tag=f"ex{k}", name=f"ex{k}")
            nc.scalar.activation(ex[:], lg[:], AF.Exp)
            sm = attn.tile([128, 1], f32, tag=f"sm{k}", name=f"sm{k}")
            nc.vector.reduce_sum(out=sm[:], in_=ex[:], axis=AX)
            rc = attn.tile([128, 1], f32, tag=f"rc{k}", name=f"rc{k}")
            nc.vector.reciprocal(out=rc[:], in_=sm[:])
            at = attn.tile([128, 4], f32, tag=f"a{k}", name=f"a{k}")
            nc.vector.tensor_scalar_mul(out=at[:], in0=ex[:], scalar1=rc[:])
            a_sb.append(at)

        # ---- scaled weights: weff[oi] = a[:,m] * wc rows; wt = weff^T ----
        weff = [
            attn.tile([128, 4 * C], f32, tag=f"weff{oi}", name=f"weff{oi}")
            for oi in range(2)
        ]
        beff = []
        for oi in range(2):
            for m in range(4):
                nc.vector.tensor_scalar_mul(
                    out=weff[oi][:, m * C : (m + 1) * C],
                    in0=wc_sb[oi][:, m * C : (m + 1) * C],
                    scalar1=a_sb[oi][:, m : m + 1],
                )
            bt = scr.tile([128, 4], f32, tag="btmp", name="btmp")
            be = attn.tile([128, 1], f32, tag=f"beff{oi}", name=f"beff{oi}")
            nc.vector.tensor_mul(bt[:], a_sb[oi][:], bc_sb[oi][:])
            nc.vector.reduce_sum(out=be[:], in_=bt[:], axis=AX)
            beff.append(be)
        wt_sb = [
            attn.tile([128, 4 * C], BF16, tag=f"wt{ci}", name=f"wt{ci}")
            for ci in range(2)
        ]
        for m in range(4):
            for oi in range(2):
                for ci in range(2):
                    psw = psM.tile([128, nw], f32, tag="ps", name="ps")
                    nc.tensor.transpose(
                        psw[:, 0:128],
                        weff[oi][:, m * C + ci * 128 : m * C + (ci + 1) * 128],
                        ident[:],
                    )
                    nc.vector.tensor_copy(
                        wt_sb[ci][:, m * C + oi * 128 : m * C + (oi + 1) * 128],
                        psw[:, 0:128],
                    )

        # ---- pass 2: out[o, s] = sum_{m,c} wt[c, o] * x_m[c, s] (+ beff) ----
        # cached pairs first (no DMA), streamed pairs re-read on the Sync ring
        # ---- diag(a_m) tiles (bf16) for the PE z-combines ----
        diag = {}
        for oi in range(2):
            for m in range(4):
                t = attn.tile([128, 128], BF16, tag=f"dg{oi}_{m}", name=f"dg{oi}_{m}")
                nc.vector.tensor_scalar(
                    out=t[:], in0=ident[:], scalar1=a_sb[oi][:, m : m + 1],
                    scalar2=0.0, op0=ALU.mult, op1=ALU.add,
                )
                diag[(oi, m)] = t

        # ---- pass 2 ----
        # z pairs first: their data is already in SBUF, so the PE starts
        # immediately once the attention weights exist while the re-read
        # prefetch keeps streaming for the GEMM pairs.
        def emit_zdiag(p):
            # out = sum_m diag(a_m) @ z_m (exact attention), beff on the drain
            for oi in range(2):
                ot = outp.tile([128, pair_w], BF16, tag="ot", name="ot")
                for nh in range(n_nh):
                    ps = psM.tile([128, nw], f32, tag="ps", name="ps")
                    for m in range(4):
                        nc.tensor.matmul(
                            ps[:], lhsT=diag[(oi, m)][:],
                            rhs=zt[(p, m, oi)][:, nh * nw : (nh + 1) * nw],
                            start=(m == 0), stop=(m == 3),
                        )
                    nc.vector.tensor_scalar_add(
                        out=ot[:, nh * nw : (nh + 1) * nw], in0=ps[:],
                        scalar1=beff[oi][:],
                    )
                nc.scalar.dma_start(
                    out=out_d[
                        oi * 128 : (oi + 1) * 128, p * pair_w : (p + 1) * pair_w
                    ],
                    in_=ot[:],
                )

        def emit_gemm(p):
            # out = wt^T x (+ beff) from re-read tiles
            xt = {(m, ci): xt2[(p, m, ci)] for m in range(4) for ci in range(2)}
            for oi in range(2):
                ot = outp.tile([128, pair_w], BF16, tag="ot", name="ot")
                for nh in range(n_nh):
                    ps = psM.tile([128, nw], f32, tag="ps", name="ps")
                    for m in range(4):
                        for ci in range(2):
                            nc.tensor.matmul(
                                ps[:],
                                lhsT=wt_sb[ci][
                                    :, m * C + oi * 128 : m * C + (oi + 1) * 128
                                ],
                                rhs=xt[(m, ci)][:, nh * nw : (nh + 1) * nw],
                                start=(m == 0 and ci == 0),
                                stop=(m == 3 and ci == 1),
                            )
                    nc.vector.tensor_scalar_add(
                        out=ot[:, nh * nw : (nh + 1) * nw], in0=ps[:],
                        scalar1=beff[oi][:],
                    )
                nc.scalar.dma_start(
                    out=out_d[
                        oi * 128 : (oi + 1) * 128, p * pair_w : (p + 1) * pair_w
                    ],
                    in_=ot[:],
                )

        for p in range(zpairs):
            emit_zdiag(p)
        for p in range(zpairs, npairs):
            emit_gemm(p)
    return nc


_CACHED = {}
LAST_RESULTS = None


def _build(wh=WH, dhalf=DHALF):
    key = (wh, dhalf)
    if key not in _CACHED:
        nc = bacc.Bacc(
            "TRN2",
            target_bir_lowering=False,
            debug=False,
            enable_asserts=False,
            num_devices=NCORES,
        )
        _emit_program(nc, wh=wh, dhalf=dhalf)
        nc.compile()
        _CACHED[key] = nc
    return _CACHED[key]


def _host_prep(Wq, bq, Wk, bk, bc, wh_pool, d):
    """Fold pooling mean + logit scale into augmented [D+1, D] q/k weights."""
    scale_q = 1.0 / (wh_pool * np.sqrt(np.float32(d)))
    wqTaug = np.concatenate(
        [(Wq * scale_q).T, (bq / np.sqrt(np.float32(d)))[None, :]], axis=0
    ).astype(np.float32)
    wkTaug = np.concatenate([(Wk / wh_pool).T, bk[None, :]], axis=0).astype(np.float32)
    bcT = np.ascontiguousarray(bc.T).astype(np.float32)
    ident = np.eye(128, dtype=np.float32)
    return wqTaug, wkTaug, bcT, ident


def kernel(m1, m2, m3, m4, Wq, bq, Wk, bk, Wc, bc, **run_kwargs):
    ms = [np.asarray(x, dtype=np.float32) for x in (m1, m2, m3, m4)]
    Wq, bq, Wk, bk, Wc, bc = (
        np.asarray(x, dtype=np.float32) for x in (Wq, bq, Wk, bk, Wc, bc)
    )
    nc = _build()
    wqTaug, wkTaug, bcT, ident = _host_prep(Wq, bq, Wk, bk, bc, WH, D)
    in_maps = []
    for p in range(NCORES):
        b, h = divmod(p, 2)
        im = {
            f"x{m}": np.ascontiguousarray(
                ms[m][b, :, h * DHALF : (h + 1) * DHALF]
            ).reshape(C, S).astype(NP_BF16)
            for m in range(4)
        }
        im.update(wqTaug=wqTaug, wkTaug=wkTaug, wc=Wc, bcT=bcT, ident=ident,
                  wcT=np.ascontiguousarray(Wc.transpose(0, 2, 1)).astype(NP_BF16))
        in_maps.append(im)
    global LAST_RESULTS
    res = run_bass_kernel_spmd(
        nc, in_maps, core_ids=list(range(NCORES)), **run_kwargs
    )
    LAST_RESULTS = res
    out = np.empty((B, C, D, W, H), np.float32)
    for p in range(NCORES):
        b, h = divmod(p, 2)
        out[b, :, h * DHALF : (h + 1) * DHALF] = (
            res.results[p]["out"].astype(np.float32).reshape(C, DHALF, W, H)
        )
    return out
